# revision 1
# baseline (speedup 1.0000x reference)
"""Multi-head causal attention with RoPE on 8 Trainium2 NeuronCores.

Sharding: core c -> (batch b = c//4, head-group g = c%4, heads 4g..4g+4).
wq/wk/wv column-sharded by head, wo row-sharded; attention fully local.
Host sums the 4 per-core partial output projections per batch.

Numerics: matmul operands bf16, all accumulation fp32 (PSUM), RoPE/trig and
softmax statistics fp32. End-to-end rel err vs fp32 reference ~2e-3.
"""
import sys
sys.path.insert(0, "/opt/trn_rl_repo")
import numpy as np
from ml_dtypes import bfloat16

import concourse.bass as bass
import concourse.tile as tile
from concourse import bacc, mybir
from concourse.bass_utils import run_bass_kernel_spmd

F = mybir.ActivationFunctionType
A = mybir.AluOpType
FP32 = mybir.dt.float32
BF16 = mybir.dt.bfloat16
I32 = mybir.dt.int32

B, D, H = 2, 1024, 16
NCORES = 8
GROUPS = 4            # head groups (cores per batch)
HL = H // GROUPS      # heads per core = 4
DK = D // H           # 64
JL = HL * DK          # local projection width = 256
ROPE_THETA = 10000.0

TWO_PI = 2 * np.pi
C1 = 6.28125                      # exact in fp32
C2 = float(np.float32(TWO_PI - C1))
C3 = float(TWO_PI - C1 - C2)
PI = float(np.pi)


def build_mha(S: int, max_phase: int = 9, reps: int = 1):
    """One SPMD program: per-core shard of the full MHA layer."""
    assert S % 512 == 0
    NT = S // 128          # 128-tiles along sequence
    NC = S // 512          # 512-chunks along sequence
    KT = D // 128          # 8 contraction tiles for projections

    nc = bacc.Bacc(None, target_bir_lowering=False, debug=False)

    xt_in = nc.declare_dram_parameter("xt", [D, S], BF16, isOutput=False)
    wq_in = nc.declare_dram_parameter("wqt", [D, JL], BF16, isOutput=False)
    wk_in = nc.declare_dram_parameter("wkt", [D, JL], BF16, isOutput=False)
    wv_in = nc.declare_dram_parameter("wvt", [D, JL], BF16, isOutput=False)
    wo_in = nc.declare_dram_parameter("wot", [JL, D], BF16, isOutput=False)
    pos_in = nc.declare_dram_parameter("pos", [1, S], I32, isOutput=False)
    ivf_in = nc.declare_dram_parameter("invfreq", [1, DK], FP32, isOutput=False)
    alt_in = nc.declare_dram_parameter("altsign", [DK, 1], FP32, isOutput=False)
    ind_in = nc.declare_dram_parameter("indicator", [2, 128], FP32, isOutput=False)
    y_out = nc.declare_dram_parameter("y", [S, D], FP32, isOutput=True)

    with tile.TileContext(nc) as tc:
        # long-lived sbuf tensors (bf16 matmul operands)
        persist = tc.alloc_tile_pool(name="persist", bufs=1)
        qTb = [persist.tile([128, S], BF16, tag=f"qTb{i}", name=f"qTb{i}") for i in range(2)]
        kTb = [persist.tile([128, S], BF16, tag=f"kTb{i}", name=f"kTb{i}") for i in range(2)]
        v_sb = persist.tile([128, NT, HL, DK + 2], BF16, tag="v")
        attnT = [persist.tile([128, S], BF16, tag=f"aT{i}", name=f"aT{i}") for i in range(2)]
        woTb = persist.tile([128, 2, D], BF16, tag="woTb")
        den_t = [persist.tile([2, S], FP32, tag=f"den{i}", name=f"den{i}") for i in range(2)]
        ind_sb = persist.tile([2, 128], FP32, tag="ind")
        cos128 = persist.tile([128, S], FP32, tag="cos128")
        sinalt128 = persist.tile([128, S], FP32, tag="sinalt128")

        nc.sync.dma_start(out=ind_sb, in_=ind_in[:, :])
        nc.sync.dma_start(out=woTb, in_=wo_in[:, :].rearrange("(t p) e -> p t e", p=128))

        for _rep in range(reps):
            nc.vector.memset(den_t[0], 0.0)
            nc.vector.memset(den_t[1], 0.0)

            # ---- Phase 1+2: projections (bf16) + RoPE ----
            with tc.tile_pool(name="proj", bufs=1) as proj, \
                 tc.tile_pool(name="proj_ps", bufs=3, space="PSUM") as pps:
                pos_i = proj.tile([1, S], I32, tag="posi")
                ivf = proj.tile([1, DK], FP32, tag="ivf")
                alt = proj.tile([DK, 1], FP32, tag="alt")
                nc.sync.dma_start(out=pos_i, in_=pos_in[:, :])
                nc.sync.dma_start(out=ivf, in_=ivf_in[:, :])
                nc.sync.dma_start(out=alt, in_=alt_in[:, :])
                xtb = proj.tile([128, KT, S], BF16, tag="xtb")
                nc.sync.dma_start(out=xtb, in_=xt_in[:, :].rearrange("(k p) s -> p k s", p=128))
                wb = {}
                for name, win in (("v", wv_in), ("k", wk_in), ("q", wq_in)):
                    wb[name] = proj.tile([128, KT, JL], BF16, tag=f"wb{name}", name=f"wb{name}")
                    nc.sync.dma_start(out=wb[name],
                                      in_=win[:, :].rearrange("(k p) j -> p k j", p=128))

                # ---- Phase 0: trig tables [128, S] fp32 (row = 64*h2 + t-slot) ----
                with tc.tile_pool(name="trig", bufs=1) as trig, \
                     tc.tile_pool(name="trig_ps", bufs=2, space="PSUM") as trig_ps, \
                     tc.tile_pool(name="trig_tmp", bufs=1) as ttmp:
                    pos_f = trig.tile([1, S], FP32, tag="posf")
                    nc.vector.tensor_copy(out=pos_f, in_=pos_i)

                    ang = trig.tile([DK, S], FP32, tag="ang")
                    for c in range(NC):
                        aps = trig_ps.tile([DK, 512], FP32, tag="angps")
                        nc.tensor.matmul(out=aps, lhsT=ivf, rhs=pos_f[:, 512 * c:512 * (c + 1)],
                                         start=True, stop=True)
                        nc.vector.tensor_copy(out=ang[:, 512 * c:512 * (c + 1)], in_=aps)

                    # range reduce: xr = ang - round(ang/2pi)*2pi, wrap to [-pi, pi]
                    xs = ttmp.tile([DK, S], FP32, tag="xs")
                    ki = ttmp.tile([DK, S], I32, tag="ki")
                    kf = xs
                    xr = ttmp.tile([DK, S], FP32, tag="xr")
                    msk = ki.bitcast(FP32)
                    nc.vector.tensor_scalar_mul(xs, ang, 1.0 / TWO_PI)
                    nc.vector.tensor_copy(out=ki, in_=xs)
                    nc.vector.tensor_copy(out=kf, in_=ki)
                    nc.vector.scalar_tensor_tensor(xr, kf, -C1, ang, op0=A.mult, op1=A.add)
                    nc.vector.scalar_tensor_tensor(xr, kf, -C2, xr, op0=A.mult, op1=A.add)
                    nc.vector.scalar_tensor_tensor(xr, kf, -C3, xr, op0=A.mult, op1=A.add)
                    nc.vector.tensor_scalar(msk, xr, PI, None, op0=A.is_gt)
                    nc.vector.scalar_tensor_tensor(xr, msk, -TWO_PI, xr, op0=A.mult, op1=A.add)

                    s64 = ttmp.tile([DK, S], FP32, tag="s64")
                    nc.scalar.activation(out=s64, in_=xr, func=F.Sin)
                    nc.vector.tensor_scalar(s64, s64, alt, None, op0=A.mult)
                    nc.sync.dma_start(out=sinalt128[0:DK, :], in_=s64)
                    nc.sync.dma_start(out=sinalt128[DK:128, :], in_=s64)
                    nc.vector.tensor_scalar(xr, xr, PI / 2, None, op0=A.add)
                    nc.vector.tensor_scalar(msk, xr, PI, None, op0=A.is_gt)
                    nc.vector.scalar_tensor_tensor(xr, msk, -TWO_PI, xr, op0=A.mult, op1=A.add)
                    c64 = ttmp.tile([DK, S], FP32, tag="c64")
                    nc.scalar.activation(out=c64, in_=xr, func=F.Sin)
                    nc.sync.dma_start(out=cos128[0:DK, :], in_=c64)
                    nc.sync.dma_start(out=cos128[DK:128, :], in_=c64)


                # v first (natural layout, bf16 out) so attention can start earliest
                for st in range(NT):
                    ps = pps.tile([128, JL], FP32, tag="vps")
                    for k in range(KT):
                        nc.tensor.matmul(out=ps, lhsT=xtb[:, k, 128 * st:128 * (st + 1)],
                                         rhs=wb["v"][:, k, :],
                                         start=(k == 0), stop=(k == KT - 1))
                    nc.vector.tensor_copy(
                        out=v_sb[:, st, :, 0:DK],
                        in_=ps[:, :].rearrange("p (h d) -> p h d", h=HL))
                for hh in range(HL):
                    nc.vector.memset(v_sb[:, :, hh, DK:DK + 1], 1.0 if hh % 2 == 0 else 0.0)
                    nc.vector.memset(v_sb[:, :, hh, DK + 1:DK + 2], 0.0 if hh % 2 == 0 else 1.0)

                # q.T, k.T per (tensor, j-tile): project into fp32 staging, rope, emit bf16
                with tc.tile_pool(name="ropep", bufs=2) as ropep:
                  for jt, name in ((0, "k"), (0, "q"), (1, "k"), (1, "q")):
                        dstpair = kTb if name == "k" else qTb
                        t32 = ropep.tile([128, S], FP32, tag="t32")
                        for sc in range(NC):
                            ps = pps.tile([128, 512], FP32, tag="projps")
                            for k in range(KT):
                                nc.tensor.matmul(
                                    out=ps,
                                    lhsT=wb[name][:, k, 128 * jt:128 * (jt + 1)],
                                    rhs=xtb[:, k, 512 * sc:512 * (sc + 1)],
                                    start=(k == 0), stop=(k == KT - 1))
                            nc.scalar.activation(out=t32[:, 512 * sc:512 * (sc + 1)],
                                                 in_=ps, func=F.Copy)
                        # RoPE: perm layout (per 64-row head block: 32 even-d rows then 32 odd-d)
                        swp = ropep.tile([128, S], FP32, tag="swp")
                        for blk in range(4):
                            src_b, dst_b = 32 * (blk ^ 1), 32 * blk
                            nc.sync.dma_start(out=swp[dst_b:dst_b + 32, :],
                                              in_=t32[src_b:src_b + 32, :])
                        tmp = ropep.tile([128, S], FP32, tag="ropetmp")
                        nc.vector.tensor_mul(tmp, t32, cos128)
                        nc.gpsimd.tensor_mul(swp, swp, sinalt128)
                        nc.gpsimd.tensor_add(dstpair[jt], tmp, swp)

            # ---- Phase 3: attention per head ----
            SCALE = 1.0 / np.sqrt(DK)
            with tc.tile_pool(name="attn_es", bufs=8) as es_pool, \
                 tc.tile_pool(name="attn_sp", bufs=2, space="PSUM") as sp_pool, \
                 tc.tile_pool(name="attn_ov", bufs=NC, space="PSUM") as ov_pool:
                for h in range(HL):
                    jt, pb = h // 2, 64 * (h % 2)
                    kTh = kTb[jt]
                    qTh = qTb[jt]
                    ov = [ov_pool.tile([DK + 2, 512], FP32, tag="ov", name=f"ov{h}_{i}") for i in range(NC)]

                    def emit_pv(mi, esr):
                        for jg in range(mi // 4, NC):
                            lo = max(512 * jg, 128 * mi)
                            hi = 512 * (jg + 1)
                            nc.tensor.matmul(
                                out=ov[jg][:, lo - 512 * jg:512],
                                lhsT=v_sb[:, mi, h, :],
                                rhs=esr[:, lo - 128 * mi:hi - 128 * mi],
                                start=(mi == 0), stop=(mi == 4 * jg + 3))

                    pending = None
                    for mi in range(NT):
                        W = S - 128 * mi
                        esr = es_pool.tile([128, S], BF16, tag="esr")
                        for cb in range(0, W, 1024):
                            cw = min(1024, W - cb)
                            sp = sp_pool.tile([128, 1024], FP32, tag="sp")
                            for sb0 in range(0, cw, 512):
                                sw = min(512, cw - sb0)
                                n0 = 128 * mi + cb + sb0
                                nc.tensor.matmul(
                                    out=sp[:, sb0:sb0 + sw],
                                    lhsT=kTh[pb:pb + DK, 128 * mi:128 * (mi + 1)],
                                    rhs=qTh[pb:pb + DK, n0:n0 + sw],
                                    start=True, stop=True)
                            nc.scalar.activation(out=esr[:, cb:cb + cw], in_=sp[:, 0:cw],
                                                 func=F.Exp, scale=SCALE)
                        # causal mask on diagonal 128 cols: keep where n-m >= 0
                        nc.gpsimd.affine_select(
                            out=esr[:, 0:128], in_=esr[:, 0:128],
                            pattern=[[1, 128]], compare_op=A.is_ge, fill=0.0,
                            base=0, channel_multiplier=-1)
                        if pending is not None:
                            emit_pv(*pending)
                        pending = (mi, esr)
                    emit_pv(*pending)
                    # unload: rows 0..63 -> attnT (bf16), rows 64..65 -> denom accum (fp32)
                    for jg in range(NC):
                        nc.vector.tensor_copy(
                            out=attnT[jt][pb:pb + DK, 512 * jg:512 * (jg + 1)],
                            in_=ov[jg][0:DK, :])
                        nc.vector.tensor_add(
                            den_t[jt][:, 512 * jg:512 * (jg + 1)],
                            den_t[jt][:, 512 * jg:512 * (jg + 1)],
                            ov[jg][DK:DK + 2, :])
                    if h % 2 == 1:
                        # normalize this j-tile now (reuses ov psum slots)
                        nc.vector.reciprocal(out=den_t[jt], in_=den_t[jt])
                        for sc in range(NC):
                            bc = ov_pool.tile([128, 512], FP32, tag="ov", name=f"bc{jt}_{sc}")
                            nc.tensor.matmul(out=bc, lhsT=ind_sb,
                                             rhs=den_t[jt][:, 512 * sc:512 * (sc + 1)],
                                             start=True, stop=True)
                            nc.vector.tensor_mul(attnT[jt][:, 512 * sc:512 * (sc + 1)],
                                                 attnT[jt][:, 512 * sc:512 * (sc + 1)], bc)

            # ---- Phase 5: output projection ----
            with tc.tile_pool(name="out_ps", bufs=3, space="PSUM") as ops, \
                 tc.tile_pool(name="out_sb", bufs=4) as osb:
                for st in range(NT):
                    for ec in range(D // 512):
                        po = ops.tile([128, 512], FP32, tag="po")
                        for jt in range(2):
                            nc.tensor.matmul(
                                out=po,
                                lhsT=attnT[jt][:, 128 * st:128 * (st + 1)],
                                rhs=woTb[:, jt, 512 * ec:512 * (ec + 1)],
                                start=(jt == 0), stop=(jt == 1))
                        yst = osb.tile([128, 512], FP32, tag="yst")
                        if ec % 2 == 0:
                            nc.scalar.activation(out=yst, in_=po, func=F.Copy)
                        else:
                            nc.vector.tensor_copy(out=yst, in_=po)
                        nc.sync.dma_start(
                            out=y_out[128 * st:128 * (st + 1), 512 * ec:512 * (ec + 1)],
                            in_=yst)

        persist.release()

    nc.compile()
    return nc


_cache = {}

def _get_program(S):
    if S not in _cache:
        _cache[S] = build_mha(S)
    return _cache[S]


def make_in_maps(x, token_positions, wq, wk, wv, wo):
    S = x.shape[1]
    invfreq = ROPE_THETA ** (-np.arange(0, DK, 2, dtype=np.float32) / DK)
    invfreq_dup = np.concatenate([invfreq, invfreq]).reshape(1, DK).astype(np.float32)
    altsign = np.concatenate([-np.ones(DK // 2), np.ones(DK // 2)]).astype(np.float32).reshape(DK, 1)
    # perm: within each 64-wide head block, evens first then odds
    blockperm = np.concatenate([np.arange(0, DK, 2), np.arange(1, DK, 2)])
    jperm = np.concatenate([64 * hh + blockperm for hh in range(HL)])
    indicator = np.zeros((2, 128), dtype=np.float32)
    indicator[0, 0:64] = 1.0
    indicator[1, 64:128] = 1.0

    in_maps = []
    for c in range(NCORES):
        b, g = c // GROUPS, c % GROUPS
        js = slice(JL * g, JL * (g + 1))
        in_maps.append({
            "xt": np.ascontiguousarray(x[b].T).astype(bfloat16),
            "wqt": np.ascontiguousarray(wq[js, :][jperm, :].T).astype(bfloat16),
            "wkt": np.ascontiguousarray(wk[js, :][jperm, :].T).astype(bfloat16),
            "wvt": np.ascontiguousarray(wv[js, :].T).astype(bfloat16),
            "wot": np.ascontiguousarray(wo[:, js].T).astype(bfloat16),
            "pos": np.asarray(token_positions[b], dtype=np.int32).reshape(1, S),
            "invfreq": invfreq_dup,
            "altsign": altsign,
            "indicator": indicator,
        })
    return in_maps


def kernel(x, token_positions, wq, wk, wv, wo):
    x = np.asarray(x, dtype=np.float32)
    token_positions = np.asarray(token_positions)
    wq = np.asarray(wq, dtype=np.float32)
    wk = np.asarray(wk, dtype=np.float32)
    wv = np.asarray(wv, dtype=np.float32)
    wo = np.asarray(wo, dtype=np.float32)
    S = x.shape[1]

    nc = _get_program(S)
    in_maps = make_in_maps(x, token_positions, wq, wk, wv, wo)
    res = run_bass_kernel_spmd(nc, in_maps, core_ids=list(range(NCORES)))
    out = np.zeros((B, S, D), dtype=np.float32)
    for c in range(NCORES):
        out[c // GROUPS] += res.results[c]["y"]
    return out



# revision 3
# speedup vs baseline: 1.1358x; 1.1358x over previous
"""Multi-head causal attention with RoPE on 8 Trainium2 NeuronCores.

Sharding: core c -> (batch b = c//4, head-group g = c%4, heads 4g..4g+4).
wq/wk/wv column-sharded by head, wo row-sharded; attention fully local.
Host sums the 4 per-core partial output projections per batch.

Numerics: matmul operands fp16, all accumulation fp32 (PSUM), RoPE trig
tables precomputed on host (fp16), softmax statistics fp32/fp16.
End-to-end rel err vs fp32 reference ~1e-3.
"""
import sys
sys.path.insert(0, "/opt/trn_rl_repo")
import numpy as np

import concourse.bass as bass
import concourse.tile as tile
from concourse import bacc, mybir
from concourse.bass_utils import run_bass_kernel_spmd

F = mybir.ActivationFunctionType
A = mybir.AluOpType
FP32 = mybir.dt.float32
FP16 = mybir.dt.float16
I32 = mybir.dt.int32

B, D, H = 2, 1024, 16
NCORES = 8
GROUPS = 4            # head groups (cores per batch)
HL = H // GROUPS      # heads per core = 4
DK = D // H           # 64
JL = HL * DK          # local projection width = 256
ROPE_THETA = 10000.0


def build_mha(S: int, max_phase: int = 9, reps: int = 1):
    """One SPMD program: per-core shard of the full MHA layer."""
    assert S % 512 == 0
    NT = S // 128          # 128-tiles along sequence
    NC = S // 512          # 512-chunks along sequence
    KT = D // 128          # 8 contraction tiles for projections

    nc = bacc.Bacc(None, target_bir_lowering=False, debug=False)

    xt_in = nc.declare_dram_parameter("xt", [D, S], FP16, isOutput=False)
    wq_in = nc.declare_dram_parameter("wqt", [D, JL], FP16, isOutput=False)
    wk_in = nc.declare_dram_parameter("wkt", [D, JL], FP16, isOutput=False)
    wv_in = nc.declare_dram_parameter("wvt", [D, JL], FP16, isOutput=False)
    wo_in = nc.declare_dram_parameter("wot", [JL, D], FP16, isOutput=False)
    cos_in = nc.declare_dram_parameter("cos64", [DK, S], FP16, isOutput=False)
    sin_in = nc.declare_dram_parameter("sinalt64", [DK, S], FP16, isOutput=False)
    ind_in = nc.declare_dram_parameter("indicator", [2, 128], FP16, isOutput=False)
    y_out = nc.declare_dram_parameter("y", [S, D], FP16, isOutput=True)

    with tile.TileContext(nc) as tc:
        # long-lived sbuf tensors (fp16 matmul operands)
        persist = tc.alloc_tile_pool(name="persist", bufs=1)
        qTb = [persist.tile([128, S], FP16, tag=f"qTb{i}", name=f"qTb{i}") for i in range(2)]
        kTb = [persist.tile([128, S], FP16, tag=f"kTb{i}", name=f"kTb{i}") for i in range(2)]
        v_sb = persist.tile([128, NT, HL, DK + 2], FP16, tag="v")
        attnT = [persist.tile([128, S], FP16, tag=f"aT{i}", name=f"aT{i}") for i in range(2)]
        woTb = persist.tile([128, 2, D], FP16, tag="woTb")
        den_t = [persist.tile([2, S], FP32, tag=f"den{i}", name=f"den{i}") for i in range(2)]
        den16 = [persist.tile([2, S], FP16, tag=f"den16_{i}", name=f"den16_{i}") for i in range(2)]
        ind_sb = persist.tile([2, 128], FP16, tag="ind")
        cos128 = persist.tile([128, S], FP16, tag="cos128")
        sinalt128 = persist.tile([128, S], FP16, tag="sinalt128")

        nc.sync.dma_start(out=ind_sb, in_=ind_in[:, :])
        nc.sync.dma_start(out=woTb, in_=wo_in[:, :].rearrange("(t p) e -> p t e", p=128))

        for _rep in range(reps):
            nc.vector.memset(den_t[0], 0.0)
            nc.vector.memset(den_t[1], 0.0)
            # trig tables: 64 rows duplicated to both head halves
            nc.sync.dma_start(out=cos128[0:DK, :], in_=cos_in[:, :])
            nc.sync.dma_start(out=cos128[DK:128, :], in_=cos_in[:, :])
            nc.sync.dma_start(out=sinalt128[0:DK, :], in_=sin_in[:, :])
            nc.sync.dma_start(out=sinalt128[DK:128, :], in_=sin_in[:, :])

            # ---- Phase 1+2: projections (fp16) + RoPE ----
            with tc.tile_pool(name="proj", bufs=1) as proj, \
                 tc.tile_pool(name="proj_ps", bufs=3, space="PSUM") as pps:
                xtb = proj.tile([128, KT, S], FP16, tag="xtb")
                # chunked along S so first projections can start early
                for sc in range(NC):
                    nc.sync.dma_start(
                        out=xtb[:, :, 512 * sc:512 * (sc + 1)],
                        in_=xt_in[:, 512 * sc:512 * (sc + 1)].rearrange(
                            "(k p) s -> p k s", p=128))
                wb = {}
                for name, win in (("v", wv_in), ("k", wk_in), ("q", wq_in)):
                    wb[name] = proj.tile([128, KT, JL], FP16, tag=f"wb{name}", name=f"wb{name}")
                    nc.sync.dma_start(out=wb[name],
                                      in_=win[:, :].rearrange("(k p) j -> p k j", p=128))

                # v first (natural layout, fp16 out) so attention can start earliest
                for st in range(NT):
                    ps = pps.tile([128, JL], FP32, tag="vps")
                    for k in range(KT):
                        nc.tensor.matmul(out=ps, lhsT=xtb[:, k, 128 * st:128 * (st + 1)],
                                         rhs=wb["v"][:, k, :],
                                         start=(k == 0), stop=(k == KT - 1))
                    nc.vector.tensor_copy(
                        out=v_sb[:, st, :, 0:DK],
                        in_=ps[:, :].rearrange("p (h d) -> p h d", h=HL))
                for hh in range(HL):
                    nc.vector.memset(v_sb[:, :, hh, DK:DK + 1], 1.0 if hh % 2 == 0 else 0.0)
                    nc.vector.memset(v_sb[:, :, hh, DK + 1:DK + 2], 0.0 if hh % 2 == 0 else 1.0)

                # q.T, k.T per (tensor, j-tile): project into fp16 staging, rope, emit fp16
                with tc.tile_pool(name="ropep", bufs=2) as ropep:
                  for jt, name in ((0, "k"), (0, "q"), (1, "k"), (1, "q")):
                        dstpair = kTb if name == "k" else qTb
                        t16 = ropep.tile([128, S], FP16, tag="t16")
                        for sc in range(NC):
                            ps = pps.tile([128, 512], FP32, tag="projps")
                            for k in range(KT):
                                nc.tensor.matmul(
                                    out=ps,
                                    lhsT=wb[name][:, k, 128 * jt:128 * (jt + 1)],
                                    rhs=xtb[:, k, 512 * sc:512 * (sc + 1)],
                                    start=(k == 0), stop=(k == KT - 1))
                            nc.scalar.activation(out=t16[:, 512 * sc:512 * (sc + 1)],
                                                 in_=ps, func=F.Copy)
                        # RoPE: perm layout (per 64-row head block: 32 even-d rows then 32 odd-d)
                        swp = ropep.tile([128, S], FP16, tag="swp")
                        for blk in range(4):
                            src_b, dst_b = 32 * (blk ^ 1), 32 * blk
                            nc.sync.dma_start(out=swp[dst_b:dst_b + 32, :],
                                              in_=t16[src_b:src_b + 32, :])
                        tmp = ropep.tile([128, S], FP16, tag="ropetmp")
                        nc.vector.tensor_mul(tmp, t16, cos128)
                        nc.vector.tensor_mul(swp, swp, sinalt128)
                        nc.vector.tensor_add(dstpair[jt], tmp, swp)

            # ---- Phase 3: attention per head ----
            SCALE = 1.0 / np.sqrt(DK)
            with tc.tile_pool(name="attn_es", bufs=8) as es_pool, \
                 tc.tile_pool(name="attn_sp", bufs=2, space="PSUM") as sp_pool, \
                 tc.tile_pool(name="attn_ov", bufs=NC, space="PSUM") as ov_pool:
                for h in range(HL):
                    jt, pb = h // 2, 64 * (h % 2)
                    kTh = kTb[jt]
                    qTh = qTb[jt]
                    ov = [ov_pool.tile([DK + 2, 512], FP32, tag="ov", name=f"ov{h}_{i}") for i in range(NC)]

                    def emit_pv(mi, esr):
                        for jg in range(mi // 4, NC):
                            lo = max(512 * jg, 128 * mi)
                            hi = 512 * (jg + 1)
                            nc.tensor.matmul(
                                out=ov[jg][:, lo - 512 * jg:512],
                                lhsT=v_sb[:, mi, h, :],
                                rhs=esr[:, lo - 128 * mi:hi - 128 * mi],
                                start=(mi == 0), stop=(mi == 4 * jg + 3))

                    pending = None
                    for mi in range(NT):
                        W = S - 128 * mi
                        esr = es_pool.tile([128, S], FP16, tag="esr")
                        for cb in range(0, W, 1024):
                            cw = min(1024, W - cb)
                            sp = sp_pool.tile([128, 1024], FP32, tag="sp")
                            for sb0 in range(0, cw, 512):
                                sw = min(512, cw - sb0)
                                n0 = 128 * mi + cb + sb0
                                nc.tensor.matmul(
                                    out=sp[:, sb0:sb0 + sw],
                                    lhsT=kTh[pb:pb + DK, 128 * mi:128 * (mi + 1)],
                                    rhs=qTh[pb:pb + DK, n0:n0 + sw],
                                    start=True, stop=True)
                            nc.scalar.activation(out=esr[:, cb:cb + cw], in_=sp[:, 0:cw],
                                                 func=F.Exp, scale=SCALE)
                        # causal mask on diagonal 128 cols: keep where n-m >= 0
                        nc.gpsimd.affine_select(
                            out=esr[:, 0:128], in_=esr[:, 0:128],
                            pattern=[[1, 128]], compare_op=A.is_ge, fill=0.0,
                            base=0, channel_multiplier=-1)
                        if pending is not None:
                            emit_pv(*pending)
                        pending = (mi, esr)
                    emit_pv(*pending)
                    # unload: rows 0..63 -> attnT (fp16), rows 64..65 -> denom accum (fp32)
                    for jg in range(NC):
                        nc.vector.tensor_copy(
                            out=attnT[jt][pb:pb + DK, 512 * jg:512 * (jg + 1)],
                            in_=ov[jg][0:DK, :])
                        nc.vector.tensor_add(
                            den_t[jt][:, 512 * jg:512 * (jg + 1)],
                            den_t[jt][:, 512 * jg:512 * (jg + 1)],
                            ov[jg][DK:DK + 2, :])
                    if h % 2 == 1:
                        # normalize this j-tile now (reuses ov psum slots)
                        with nc.allow_low_precision(reason="1/den fits fp16; rel err ~1e-3 ok"):
                            nc.vector.reciprocal(out=den16[jt], in_=den_t[jt])
                        for sc in range(NC):
                            bc = ov_pool.tile([128, 512], FP32, tag="ov", name=f"bc{jt}_{sc}")
                            nc.tensor.matmul(out=bc, lhsT=ind_sb,
                                             rhs=den16[jt][:, 512 * sc:512 * (sc + 1)],
                                             start=True, stop=True)
                            nc.vector.tensor_mul(attnT[jt][:, 512 * sc:512 * (sc + 1)],
                                                 attnT[jt][:, 512 * sc:512 * (sc + 1)], bc)

            # ---- Phase 5: output projection ----
            with tc.tile_pool(name="out_ps", bufs=3, space="PSUM") as ops, \
                 tc.tile_pool(name="out_sb", bufs=4) as osb:
                for st in range(NT):
                    for ec in range(D // 512):
                        po = ops.tile([128, 512], FP32, tag="po")
                        for jt in range(2):
                            nc.tensor.matmul(
                                out=po,
                                lhsT=attnT[jt][:, 128 * st:128 * (st + 1)],
                                rhs=woTb[:, jt, 512 * ec:512 * (ec + 1)],
                                start=(jt == 0), stop=(jt == 1))
                        yst = osb.tile([128, 512], FP16, tag="yst")
                        if ec % 2 == 0:
                            nc.scalar.activation(out=yst, in_=po, func=F.Copy)
                        else:
                            nc.vector.tensor_copy(out=yst, in_=po)
                        nc.sync.dma_start(
                            out=y_out[128 * st:128 * (st + 1), 512 * ec:512 * (ec + 1)],
                            in_=yst)

        persist.release()

    nc.compile()
    return nc


_cache = {}

def _get_program(S):
    if S not in _cache:
        _cache[S] = build_mha(S)
    return _cache[S]


def make_in_maps(x, token_positions, wq, wk, wv, wo):
    S = x.shape[1]
    f16 = np.float16
    invfreq = ROPE_THETA ** (-np.arange(0, DK, 2, dtype=np.float64) / DK)  # [32]
    # perm: within each 64-wide head block, evens first then odds
    blockperm = np.concatenate([np.arange(0, DK, 2), np.arange(1, DK, 2)])
    jperm = np.concatenate([64 * hh + blockperm for hh in range(HL)])
    indicator = np.zeros((2, 128), dtype=f16)
    indicator[0, 0:64] = 1.0
    indicator[1, 64:128] = 1.0

    # trig tables per batch: [64, S] (rows 0..31 = -sin/cos for evens? see layout)
    # row r in [0,32): freq invfreq[r]; rows [32,64): same freqs (odd slots)
    pos = np.asarray(token_positions, dtype=np.float64)  # [B, S]
    tables = []
    for b in range(B):
        ang = pos[b][None, :] * invfreq[:, None]          # [32, S]
        cos = np.cos(ang)
        sin = np.sin(ang)
        cos64 = np.concatenate([cos, cos], axis=0).astype(f16)       # [64, S]
        sinalt = np.concatenate([-sin, sin], axis=0).astype(f16)     # [64, S]
        tables.append((np.ascontiguousarray(cos64), np.ascontiguousarray(sinalt)))

    in_maps = []
    for c in range(NCORES):
        b, g = c // GROUPS, c % GROUPS
        js = slice(JL * g, JL * (g + 1))
        cos64, sinalt = tables[b]
        in_maps.append({
            "xt": np.ascontiguousarray(x[b].T).astype(f16),
            "wqt": np.ascontiguousarray(wq[js, :][jperm, :].T).astype(f16),
            "wkt": np.ascontiguousarray(wk[js, :][jperm, :].T).astype(f16),
            "wvt": np.ascontiguousarray(wv[js, :].T).astype(f16),
            "wot": np.ascontiguousarray(wo[:, js].T).astype(f16),
            "cos64": cos64,
            "sinalt64": sinalt,
            "indicator": indicator,
        })
    return in_maps


def kernel(x, token_positions, wq, wk, wv, wo):
    x = np.asarray(x, dtype=np.float32)
    token_positions = np.asarray(token_positions)
    wq = np.asarray(wq, dtype=np.float32)
    wk = np.asarray(wk, dtype=np.float32)
    wv = np.asarray(wv, dtype=np.float32)
    wo = np.asarray(wo, dtype=np.float32)
    S = x.shape[1]

    nc = _get_program(S)
    in_maps = make_in_maps(x, token_positions, wq, wk, wv, wo)
    res = run_bass_kernel_spmd(nc, in_maps, core_ids=list(range(NCORES)))
    out = np.zeros((B, S, D), dtype=np.float32)
    for c in range(NCORES):
        out[c // GROUPS] += res.results[c]["y"].astype(np.float32)
    return out
